# revision 1
# baseline (speedup 1.0000x reference)
"""Trainium2 Bass kernel for nn_Attention_54614804136573 (topk_masking).

Sharding: 8 cores = 4 batches x 2 head-groups (8 heads each). Each core gets
its batch's full x (columns rotated so its own 8 head-chunks come first),
computes the token-importance mask redundantly, runs its 8 heads of attention,
and produces a partial to_out product for its 1024-wide d-slice. The host sums
the two partials per batch and adds bo.
"""

import sys

sys.path.insert(0, "/opt/trn_rl_repo")

import numpy as np
import ml_dtypes

import concourse.mybir as mybir
import concourse.tile as tile
from concourse import bacc, bass_utils
from concourse.masks import make_identity
from concourse.tile import add_dep_helper

B = 4
N = 1024
C = 128
D = 2048
NCHUNK = 16  # d-chunks of 128 (= patch positions = heads)
HPC = 8  # heads per core
MASK_NUM = 25
SCALE = 64.0 ** -0.5  # 0.125

F32 = mybir.dt.float32
F32R = mybir.dt.float32r
BF16 = mybir.dt.bfloat16
U32 = mybir.dt.uint32
Exp = mybir.ActivationFunctionType.Exp
Ident = mybir.ActivationFunctionType.Identity
Copy = mybir.ActivationFunctionType.Copy
Copy = mybir.ActivationFunctionType.Copy
NEG_BIG = -1e30


def _body(tc, xc, wq_d, wk_d, wv_d, bq_d, bk_d, bv_d, wtc_d, wo_d, outT_d):
    nc = tc.nc
    mscr = nc.dram_tensor("mscr", (N,), F32, kind="Internal").ap()
    dscr = nc.dram_tensor("dscr", (HPC, N), F32, kind="Internal").ap()

    with (
        tc.tile_pool(name="consts", bufs=1) as consts,
        tc.tile_pool(name="persist", bufs=1) as persist,
    ):
        # ---- constants ----
        ident_ld = consts.tile([128, 128], F32)
        make_identity(nc, ident_ld)
        ident = consts.tile([128, 128], F32R)
        nc.vector.tensor_copy(ident, ident_ld)
        identb = consts.tile([128, 128], BF16)
        make_identity(nc, identb)
        ones_pv = consts.tile([128, 32], BF16)
        nc.vector.memset(ones_pv, 1.0)
        ones_k1_ld = consts.tile([1, 128], F32)
        nc.vector.memset(ones_k1_ld, 1.0)
        ones_k1 = consts.tile([1, 128], F32R)
        nc.vector.tensor_copy(ones_k1, ones_k1_ld)

        # ---- persistent activations ----
        qT = persist.tile([128, HPC, N], F32R)  # [c', h, n] 4 MB
        kT = persist.tile([128, HPC, N], F32R)  # 4 MB
        vnat = persist.tile([128, HPC, 8, C], BF16)  # [j, h, jt, c] 2 MB
        mask_col = persist.tile([128, 8], F32)
        scale_col = persist.tile([128, 8], F32)

        # ================= phase 1: transpose x, logits, mask, QKV =========
        with (
            tc.tile_pool(name="ph1", bufs=2) as ph1,
            tc.tile_pool(name="ph1_vt", bufs=2) as ph1_vt,
            tc.tile_pool(name="ph1big", bufs=1) as ph1big,
            tc.tile_pool(name="mrows", bufs=1) as mrows,
            tc.tile_pool(name="tp_psum", bufs=2, space="PSUM") as tp_psum,
            tc.tile_pool(name="mm_psum", bufs=2, space="PSUM") as mm_psum,
            tc.tile_pool(name="lg_psum", bufs=1, space="PSUM") as lg_psum,
        ):
            xT = ph1big.tile([128, NCHUNK, N], F32R)  # [c, k, n] 8 MB

            for nt in range(8):
                x_nat = ph1.tile([128, D], F32R)
                eng = nc.sync if nt % 2 == 0 else nc.scalar
                step = D // 2
                for dh in range(2):
                    eng.dma_start(
                        out=x_nat[:, dh * step : (dh + 1) * step],
                        in_=xc[nt * 128 : (nt + 1) * 128,
                               dh * step : (dh + 1) * step],
                    )
                for kg in range(4):
                    pt4 = tp_psum.tile([128, 4, 128], F32R, tag="pt4")
                    for dk in range(4):
                        k = kg * 4 + dk
                        nc.tensor.transpose(
                            pt4[:, dk, :], x_nat[:, k * 128 : (k + 1) * 128], ident
                        )
                    dst = xT[:, kg * 4 : kg * 4 + 4, nt * 128 : (nt + 1) * 128]
                    if nt % 2 == 0:
                        nc.vector.tensor_copy(dst, pt4)
                    else:
                        nc.scalar.activation(out=dst, in_=pt4, func=Copy)

            # weight loads: emitted after x so they don't block the x queues
            wq_ld = consts.tile([C, C], F32)
            nc.scalar.dma_start(out=wq_ld, in_=wq_d)
            wq_sb = consts.tile([C, C], F32R)
            nc.vector.tensor_copy(wq_sb, wq_ld)
            wk_ld = consts.tile([C, C], F32)
            nc.scalar.dma_start(out=wk_ld, in_=wk_d)
            wk_sb = consts.tile([C, C], F32R)
            nc.vector.tensor_copy(wk_sb, wk_ld)
            wv_ld = consts.tile([C, C], F32)
            nc.scalar.dma_start(out=wv_ld, in_=wv_d)
            wv_sb = consts.tile([C, C], F32R)
            nc.vector.tensor_copy(wv_sb, wv_ld)
            bq_sb = consts.tile([C, 1], F32)
            nc.scalar.dma_start(out=bq_sb, in_=bq_d)
            bk_sb = consts.tile([C, 1], F32)
            nc.scalar.dma_start(out=bk_sb, in_=bk_d)
            bv_sb = consts.tile([C, 1], F32)
            nc.scalar.dma_start(out=bv_sb, in_=bv_d)
            wtc_ld = consts.tile([C, 1], F32)
            nc.scalar.dma_start(out=wtc_ld, in_=wtc_d)
            wtc_sb = consts.tile([C, 1], F32R)
            nc.vector.tensor_copy(wtc_sb, wtc_ld)

            # logits[n] = sum_k xT[:, k, n] . wtc   (wtc = (Wl@Wq)/16)
            lg = lg_psum.tile([1, N], F32)
            for half in range(2):
                for k in range(NCHUNK):
                    nc.tensor.matmul(
                        lg[:, half * 512 : (half + 1) * 512],
                        wtc_sb,
                        xT[:, k, half * 512 : (half + 1) * 512],
                        start=(k == 0),
                        stop=(k == NCHUNK - 1),
                    )

            # ---- mask: softmax over tokens + snap all but 25 smallest to 1
            smrow = mrows.tile([1, N], F32)
            ssum = mrows.tile([1, 1], F32)
            nc.scalar.activation(out=smrow, in_=lg, func=Exp, accum_out=ssum)
            srecip = mrows.tile([1, 1], F32)
            nc.vector.reciprocal(srecip, ssum)
            nc.vector.tensor_scalar_mul(smrow, smrow, srecip)
            negrow = mrows.tile([1, N], F32)
            nc.vector.tensor_scalar_mul(negrow, lg, -1.0)
            scratch = mrows.tile([1, N], F32)
            nc.vector.tensor_copy(scratch, negrow)
            m8 = mrows.tile([1, 8], F32)
            for _ in range(3):
                nc.vector.max(out=m8, in_=scratch)
                nc.vector.match_replace(
                    out=scratch, in_to_replace=m8, in_values=scratch,
                    imm_value=NEG_BIG,
                )
            nc.vector.max(out=m8, in_=scratch)  # m8[0,0] = 25th largest of -L
            sel = mrows.tile([1, N], U32)
            nc.vector.tensor_scalar(
                sel, negrow, m8[:, 0:1], None, op0=mybir.AluOpType.is_lt
            )
            onesrow = mrows.tile([1, N], F32)
            nc.vector.memset(onesrow, 1.0)
            nc.vector.copy_predicated(smrow, sel, onesrow)
            # row [1, 1024] -> column-major [128, 8] (partition = token % 128)
            w_inst = nc.sync.dma_start(out=mscr, in_=smrow)
            r_inst = nc.sync.dma_start(
                out=mask_col, in_=mscr.rearrange("(t j) -> j t", j=128)
            )
            add_dep_helper(r_inst.ins, w_inst.ins, sync=True, reason="mask RAW via dram")
            nc.vector.tensor_scalar_mul(scale_col, mask_col, SCALE)

            # ---- Q/K projections (transposed layouts) ----
            for h in range(HPC):
                for w_sb, b_sb, dstT in ((wq_sb, bq_sb, qT), (wk_sb, bk_sb, kT)):
                    pp = mm_psum.tile([128, N], F32)
                    for half in range(2):
                        nc.tensor.matmul(
                            pp[:, half * 512 : (half + 1) * 512],
                            w_sb,
                            xT[:, h, half * 512 : (half + 1) * 512],
                            start=True,
                            stop=True,
                        )
                    nc.scalar.activation(
                        out=dstT[:, h, :], in_=pp, func=Ident, bias=b_sb
                    )
            # ---- V projections + transpose to natural layout (needs mask) --
            for h in range(HPC):
                vT_h = ph1_vt.tile([128, N], BF16)
                pp = mm_psum.tile([128, N], F32)
                for half in range(2):
                    nc.tensor.matmul(
                        pp[:, half * 512 : (half + 1) * 512],
                        wv_sb,
                        xT[:, h, half * 512 : (half + 1) * 512],
                        start=True,
                        stop=True,
                    )
                nc.scalar.activation(out=vT_h, in_=pp, func=Ident, bias=bv_sb)
                for jtg in range(2):
                    pv4 = tp_psum.tile([128, 4, 128], BF16, tag="pt4")
                    for dj in range(4):
                        jt = jtg * 4 + dj
                        nc.tensor.transpose(
                            pv4[:, dj, :], vT_h[:, jt * 128 : (jt + 1) * 128],
                            identb,
                        )
                    mslice = mask_col[:, jtg * 4 : (jtg + 1) * 4]
                    nc.vector.tensor_tensor(
                        out=vnat[:, h, jtg * 4 : (jtg + 1) * 4, :],
                        in0=pv4,
                        in1=mslice.unsqueeze(-1).broadcast_to([128, 4, 128]),
                        op=mybir.AluOpType.mult,
                    )

        # ================= phase 2: attention ==============================
        with tc.tile_pool(name="ph2big", bufs=1) as ph2big:
            outT_sb = ph2big.tile([128, HPC, N], BF16)  # [c, h, i] 2 MB
            woT_sb = ph2big.tile([128, HPC, D], BF16)  # [d, h-chunk, o] 4 MB
            for h in range(HPC):
                nc.sync.dma_start(
                    out=woT_sb[:, h, :], in_=wo_d[h * 128 : (h + 1) * 128, :]
                )

            attn_pools = (
                tc.tile_pool(name="pexp", bufs=8),
                tc.tile_pool(name="dvp", bufs=2),
                tc.tile_pool(name="st_psum", bufs=2, space="PSUM"),
                tc.tile_pool(name="ot_psum", bufs=1, space="PSUM"),
                tc.tile_pool(name="dn_psum", bufs=1, space="PSUM"),
            )
            pexp, dvp, st_psum, ot_psum, dn_psum = (
                p.__enter__() for p in attn_pools
            )
            for h in range(HPC):
                ot = ot_psum.tile([128, N], F32)
                dn4 = dn_psum.tile([128, N], F32, tag="dn")

                def emit_pv(jt, pexp_t, ot=ot, h=h):
                    for half in range(2):
                        nc.tensor.matmul(
                            ot[:, half * 512 : (half + 1) * 512],
                            vnat[:, h, jt, :],
                            pexp_t[:, half * 512 : (half + 1) * 512],
                            start=(jt == 0),
                            stop=(jt == 7),
                        )

                def emit_dens(blk_exps, blk, dn4=dn4):
                    # 4 col-groups back-to-back -> concurrent on the PE array
                    for half in range(2):
                        for jt, pexp_t in blk_exps:
                            g = jt % 4
                            nc.tensor.matmul(
                                dn4[32 * g : 32 * (g + 1),
                                    half * 512 : (half + 1) * 512],
                                ones_pv,
                                pexp_t[:, half * 512 : (half + 1) * 512],
                                start=(blk == 0),
                                stop=(blk == 1),
                                tile_position=(0, 32 * g),
                            )

                pending = None  # (jt, pexp tile) waiting for its PV emission
                blk_exps = []
                for jt in range(8):
                    st = st_psum.tile([128, N], F32)
                    for half in range(2):
                        nc.tensor.matmul(
                            st[:, half * 512 : (half + 1) * 512],
                            kT[:, h, jt * 128 : (jt + 1) * 128],
                            qT[:, h, half * 512 : (half + 1) * 512],
                            start=True,
                            stop=True,
                        )
                    pexp_t = pexp.tile([128, N], BF16)
                    nc.scalar.activation(
                        out=pexp_t, in_=st, func=Exp,
                        scale=scale_col[:, jt : jt + 1],
                    )
                    if jt == 4:
                        emit_dens(blk_exps, 0)
                        blk_exps = []
                    if pending is not None:
                        emit_pv(*pending)
                    pending = (jt, pexp_t)
                    blk_exps.append((jt, pexp_t))
                emit_pv(*pending)
                emit_dens(blk_exps, 1)
                nc.vector.tensor_copy(outT_sb[:, h, :], ot)
                rrow = dvp.tile([1, N], F32)
                dsb4 = dvp.tile([128, N], F32)
                nc.vector.tensor_copy(dsb4, dn4)
                nc.gpsimd.dma_start(out=rrow, in_=dsb4[0:1, :])
                for g in range(1, 4):
                    nc.gpsimd.dma_start(
                        out=rrow, in_=dsb4[32 * g : 32 * g + 1, :],
                        accum_op=mybir.AluOpType.add,
                    )
                nc.vector.reciprocal(rrow, rrow)
                w_i = nc.sync.dma_start(out=dscr[h, :], in_=rrow)
                rb_sb = dvp.tile([128, N], F32)
                r_i = nc.sync.dma_start(
                    out=rb_sb, in_=dscr[h, :].partition_broadcast(128)
                )
                add_dep_helper(r_i.ins, w_i.ins, sync=True, reason="recip RAW")
                nc.vector.tensor_mul(
                    outT_sb[:, h, :], outT_sb[:, h, :], rb_sb
                )

            # ============= phase 4: to_out partial =========================
            # fo shares the st_psum slots (same shape) so Wo accumulation can
            # begin as soon as the last exp frees an ST slot -- no pool
            # transition barrier.
            with tc.tile_pool(name="fout", bufs=3) as fout_pool:
                def finish_oc(oc, fo):
                    for half in range(2):
                        nc.tensor.matmul(
                            fo[:, half * 512 : (half + 1) * 512],
                            woT_sb[:, HPC - 1, oc * 128 : (oc + 1) * 128],
                            outT_sb[:, HPC - 1, half * 512 : (half + 1) * 512],
                            start=False,
                            stop=True,
                        )
                    fout = fout_pool.tile([128, N], F32)
                    nc.vector.tensor_copy(fout, fo)
                    for sh in range(2):
                        eng = nc.sync if sh == 0 else nc.scalar
                        eng.dma_start(
                            out=outT_d[oc * 128 : (oc + 1) * 128,
                                       sh * 512 : (sh + 1) * 512],
                            in_=fout[:, sh * 512 : (sh + 1) * 512],
                        )

                pending_oc = None
                for oc in range(16):
                    fo = st_psum.tile([128, N], F32, tag="st")
                    for half in range(2):
                        for h in range(HPC - 1):
                            nc.tensor.matmul(
                                fo[:, half * 512 : (half + 1) * 512],
                                woT_sb[:, h, oc * 128 : (oc + 1) * 128],
                                outT_sb[:, h, half * 512 : (half + 1) * 512],
                                start=(h == 0),
                                stop=False,
                            )
                    if pending_oc is not None:
                        finish_oc(*pending_oc)
                    pending_oc = (oc, fo)
                finish_oc(*pending_oc)

            for p in reversed(attn_pools):
                p.__exit__(None, None, None)


_CACHE = {}


def _get_module():
    if "nc" in _CACHE:
        return _CACHE["nc"]
    nc = bacc.Bacc("TRN2", target_bir_lowering=False, debug=False, num_devices=8)
    xc = nc.dram_tensor("xc", (N, D), F32R, kind="ExternalInput").ap()
    wq_d = nc.dram_tensor("wqT", (C, C), F32, kind="ExternalInput").ap()
    wk_d = nc.dram_tensor("wkT", (C, C), F32, kind="ExternalInput").ap()
    wv_d = nc.dram_tensor("wvT", (C, C), F32, kind="ExternalInput").ap()
    bq_d = nc.dram_tensor("bq", (C, 1), F32, kind="ExternalInput").ap()
    bk_d = nc.dram_tensor("bk", (C, 1), F32, kind="ExternalInput").ap()
    bv_d = nc.dram_tensor("bv", (C, 1), F32, kind="ExternalInput").ap()
    wtc_d = nc.dram_tensor("wtc", (C, 1), F32, kind="ExternalInput").ap()
    wo_d = nc.dram_tensor("woT", (HPC * C, D), BF16, kind="ExternalInput").ap()
    outT_d = nc.dram_tensor("outT", (D, N), F32, kind="ExternalOutput").ap()

    with tile.TileContext(nc) as tc:
        _body(tc, xc, wq_d, wk_d, wv_d, bq_d, bk_d, bv_d, wtc_d, wo_d, outT_d)
    nc.compile()
    _CACHE["nc"] = nc
    return nc


def make_in_maps(x, Wq, bq, Wk, bk, Wv, bv, Wl, bl, Wo, bo):
    x = np.ascontiguousarray(np.asarray(x, np.float32))
    Wq = np.asarray(Wq, np.float32)
    Wk = np.asarray(Wk, np.float32)
    Wv = np.asarray(Wv, np.float32)
    Wl = np.asarray(Wl, np.float32)
    Wo = np.asarray(Wo, np.float32)
    we = (Wl[0] @ Wq) / float(NCHUNK)  # (128,) logits weight per chunk
    common = {
        "wqT": np.ascontiguousarray(Wq.T),
        "wkT": np.ascontiguousarray(Wk.T),
        "wvT": np.ascontiguousarray(Wv.T),
        "bq": np.asarray(bq, np.float32).reshape(C, 1),
        "bk": np.asarray(bk, np.float32).reshape(C, 1),
        "bv": np.asarray(bv, np.float32).reshape(C, 1),
        "wtc": we.astype(np.float32).reshape(C, 1),
    }
    woT = np.ascontiguousarray(Wo.T)  # (d, o)
    woT_half = [
        woT[0:1024, :].astype(ml_dtypes.bfloat16),
        woT[1024:2048, :].astype(ml_dtypes.bfloat16),
    ]
    in_maps = []
    for core in range(8):
        b, g = divmod(core, 2)
        xb = x[b]
        xcore = xb if g == 0 else np.ascontiguousarray(
            np.concatenate([xb[:, 1024:], xb[:, :1024]], axis=1)
        )
        in_maps.append({"xc": xcore, "woT": woT_half[g], **common})
    return in_maps


def run_spmd(in_maps, trace=False, **kw):
    nc = _get_module()
    return bass_utils.run_bass_kernel_spmd(
        nc, in_maps, core_ids=list(range(8)), trace=trace, **kw
    )


def gather(results, bo):
    bo = np.asarray(bo, np.float32)
    out = np.empty((B, N, D), np.float32)
    for b in range(B):
        p0 = results[2 * b]["outT"].T
        p1 = results[2 * b + 1]["outT"].T
        out[b] = p0 + p1 + bo
    return out


def kernel(x, Wq, bq, Wk, bk, Wv, bv, Wl, bl, Wo, bo, stage=None, **_unused):
    in_maps = make_in_maps(x, Wq, bq, Wk, bk, Wv, bv, Wl, bl, Wo, bo)
    try:
        res = run_spmd(in_maps)
    except Exception:
        # transient device/runtime hiccup: retry once after a short pause
        import time as _time

        _time.sleep(2.0)
        res = run_spmd(in_maps)
    return gather(res.results, bo)



# revision 6
# speedup vs baseline: 1.0139x; 1.0139x over previous
"""Trainium2 Bass kernel for nn_Attention_54614804136573 (topk_masking).

Sharding: 8 cores = 4 batches x 2 head-groups (8 heads each). Each core gets
its batch's x pre-transposed on host to [d, n] (rows rotated so its own 8
head-chunks come first), computes the token-importance mask redundantly,
runs its 8 heads of attention, and produces a partial to_out product for all
2048 output channels. The host sums the two partials per batch and adds bo.

Key device-side structure (v2):
  - xT chunks are DMA'd directly (host pre-transposed), own chunks first on
    two queues; Q/K/V projections + logits consume each chunk as it lands.
  - softmax denominator is computed off the PE: pairwise bf16 tree-adds of
    the exp tiles on DVE, then a gpsimd partition_all_reduce (f32 accum,
    result broadcast to all partitions), reciprocal + fused normalize on DVE.
  - V is copied unmasked into its natural layout early; the token mask is
    applied with one in-place DVE multiply once available.
  - to_out accumulates per output-chunk over this core's 8 head-chunks with
    h-outer loop order so consecutive matmuls share stationary weights.
"""

import sys

sys.path.insert(0, "/opt/trn_rl_repo")

import numpy as np
import ml_dtypes

import concourse.mybir as mybir
import concourse.tile as tile
from concourse import bacc, bass_utils
from concourse.masks import make_identity
from concourse.tile import add_dep_helper

B = 4
N = 1024
C = 128
D = 2048
NCHUNK = 16  # d-chunks of 128 (= patch positions = heads)
HPC = 8  # heads per core
MASK_NUM = 25
SCALE = 64.0 ** -0.5  # 0.125

F32 = mybir.dt.float32
F32R = mybir.dt.float32r
BF16 = mybir.dt.bfloat16
U32 = mybir.dt.uint32
Exp = mybir.ActivationFunctionType.Exp
Ident = mybir.ActivationFunctionType.Identity
Copy = mybir.ActivationFunctionType.Copy
NEG_BIG = -1e30
Add = mybir.AluOpType.add
Mult = mybir.AluOpType.mult


def _body(tc, xTd, wq_d, wk_d, wv_d, bq_d, bk_d, bv_d, wtc_d, wo_d, outT_d):
    nc = tc.nc
    import concourse.bass_isa as bass_isa

    mscr = nc.dram_tensor("mscr", (N,), F32, kind="Internal").ap()

    with (
        tc.tile_pool(name="consts", bufs=1) as consts,
        tc.tile_pool(name="persist", bufs=1) as persist,
    ):
        # ---- constants ----
        identb = consts.tile([128, 128], BF16)
        make_identity(nc, identb)

        # ---- persistent activations ----
        qT = persist.tile([128, HPC, N], F32R)  # [c', h, n] 4 MB
        kT = persist.tile([128, HPC, N], F32R)  # 4 MB
        vnat = persist.tile([128, HPC, 8, C], BF16)  # [j, h, jt, c] 2 MB
        mask_col = persist.tile([128, 8], F32)
        scale_col = persist.tile([128, 8], F32)
        woT_sb = persist.tile([128, HPC, D], BF16)  # [d, h-chunk, o] 4 MB

        # ================= phase 1: stream xT chunks, QKV, logits, mask ====
        with (
            tc.tile_pool(name="ph1_vt", bufs=2) as ph1_vt,
            tc.tile_pool(name="ph1big", bufs=1) as ph1big,
            tc.tile_pool(name="mrows", bufs=1) as mrows,
            tc.tile_pool(name="tp_psum", bufs=2, space="PSUM") as tp_psum,
            tc.tile_pool(name="mm_psum", bufs=2, space="PSUM") as mm_psum,
            tc.tile_pool(name="lg_psum", bufs=1, space="PSUM") as lg_psum,
        ):
            xT = ph1big.tile([128, NCHUNK, N], F32R)  # [c, k, n] 8 MB

            # own chunks 0..7 first, split across the sync + vector queues;
            # partner chunks 8..15 after (only needed for the mask logits)
            for k in range(NCHUNK):
                eng = nc.sync if k % 2 == 0 else nc.scalar
                eng.dma_start(
                    out=xT[:, k, :], in_=xTd[k * 128 : (k + 1) * 128, :]
                )

            # weights: gpsimd queue (x occupies sync+scalar)
            wq_ld = consts.tile([C, C], F32)
            nc.gpsimd.dma_start(out=wq_ld, in_=wq_d)
            wq_sb = consts.tile([C, C], F32R)
            nc.vector.tensor_copy(wq_sb, wq_ld)
            wk_ld = consts.tile([C, C], F32)
            nc.gpsimd.dma_start(out=wk_ld, in_=wk_d)
            wk_sb = consts.tile([C, C], F32R)
            nc.vector.tensor_copy(wk_sb, wk_ld)
            wv_ld = consts.tile([C, C], F32)
            nc.gpsimd.dma_start(out=wv_ld, in_=wv_d)
            wv_sb = consts.tile([C, C], F32R)
            nc.vector.tensor_copy(wv_sb, wv_ld)
            bq_sb = consts.tile([C, 1], F32)
            nc.gpsimd.dma_start(out=bq_sb, in_=bq_d)
            bk_sb = consts.tile([C, 1], F32)
            nc.gpsimd.dma_start(out=bk_sb, in_=bk_d)
            bv_sb = consts.tile([C, 1], F32)
            nc.gpsimd.dma_start(out=bv_sb, in_=bv_d)
            wtc_ld = consts.tile([C, 1], F32)
            nc.gpsimd.dma_start(out=wtc_ld, in_=wtc_d)
            wtc_sb = consts.tile([C, 1], F32R)
            nc.vector.tensor_copy(wtc_sb, wtc_ld)
            # Wo (4 MB bf16) on the gpsimd queue, after the small weights
            for h in range(HPC):
                nc.gpsimd.dma_start(
                    out=woT_sb[:, h, :], in_=wo_d[h * 128 : (h + 1) * 128, :]
                )

            lg = lg_psum.tile([1, N], F32)

            # per own chunk: Q/K/V projections + logits contribution
            for h in range(HPC):
                for w_sb, b_sb, dstT in ((wq_sb, bq_sb, qT), (wk_sb, bk_sb, kT)):
                    pp = mm_psum.tile([128, N], F32)
                    for half in range(2):
                        nc.tensor.matmul(
                            pp[:, half * 512 : (half + 1) * 512],
                            w_sb,
                            xT[:, h, half * 512 : (half + 1) * 512],
                            start=True,
                            stop=True,
                        )
                    nc.scalar.activation(
                        out=dstT[:, h, :], in_=pp, func=Ident, bias=b_sb
                    )
                # V: project, add bias, transpose to natural [j, c] layout
                # (unmasked for now; mask applied in-place later)
                vT_h = ph1_vt.tile([128, N], BF16)
                pp = mm_psum.tile([128, N], F32)
                for half in range(2):
                    nc.tensor.matmul(
                        pp[:, half * 512 : (half + 1) * 512],
                        wv_sb,
                        xT[:, h, half * 512 : (half + 1) * 512],
                        start=True,
                        stop=True,
                    )
                nc.scalar.activation(out=vT_h, in_=pp, func=Ident, bias=bv_sb)
                for jtg in range(2):
                    pv4 = tp_psum.tile([128, 4, 128], BF16, tag="pt4")
                    for dj in range(4):
                        jt = jtg * 4 + dj
                        nc.tensor.transpose(
                            pv4[:, dj, :], vT_h[:, jt * 128 : (jt + 1) * 128],
                            identb,
                        )
                    nc.scalar.activation(
                        out=vnat[:, h, jtg * 4 : (jtg + 1) * 4, :],
                        in_=pv4, func=Copy,
                    )
                # logits contribution for this chunk
                for half in range(2):
                    nc.tensor.matmul(
                        lg[:, half * 512 : (half + 1) * 512],
                        wtc_sb,
                        xT[:, h, half * 512 : (half + 1) * 512],
                        start=(h == 0),
                        stop=False,
                    )
            # partner chunks: logits only
            for k in range(HPC, NCHUNK):
                for half in range(2):
                    nc.tensor.matmul(
                        lg[:, half * 512 : (half + 1) * 512],
                        wtc_sb,
                        xT[:, k, half * 512 : (half + 1) * 512],
                        start=False,
                        stop=(k == NCHUNK - 1),
                    )

            # ---- mask: softmax over tokens + snap all but 25 smallest to 1
            smrow = mrows.tile([1, N], F32)
            ssum = mrows.tile([1, 1], F32)
            nc.scalar.activation(out=smrow, in_=lg, func=Exp, accum_out=ssum)
            srecip = mrows.tile([1, 1], F32)
            nc.vector.reciprocal(srecip, ssum)
            nc.vector.tensor_scalar_mul(smrow, smrow, srecip)
            negrow = mrows.tile([1, N], F32)
            nc.vector.tensor_scalar_mul(negrow, lg, -1.0)
            scratch = mrows.tile([1, N], F32)
            nc.vector.tensor_copy(scratch, negrow)
            m8 = mrows.tile([1, 8], F32)
            for _ in range(3):
                nc.vector.max(out=m8, in_=scratch)
                nc.vector.match_replace(
                    out=scratch, in_to_replace=m8, in_values=scratch,
                    imm_value=NEG_BIG,
                )
            nc.vector.max(out=m8, in_=scratch)  # m8[0,0] = 25th largest of -L
            sel = mrows.tile([1, N], U32)
            nc.vector.tensor_scalar(
                sel, negrow, m8[:, 0:1], None, op0=mybir.AluOpType.is_lt
            )
            onesrow = mrows.tile([1, N], F32)
            nc.vector.memset(onesrow, 1.0)
            nc.vector.copy_predicated(smrow, sel, onesrow)
            # row [1, 1024] -> column-major [128, 8] (partition = token % 128)
            w_inst = nc.sync.dma_start(out=mscr, in_=smrow)
            r_inst = nc.sync.dma_start(
                out=mask_col, in_=mscr.rearrange("(t j) -> j t", j=128)
            )
            add_dep_helper(r_inst.ins, w_inst.ins, sync=True, reason="mask RAW via dram")
            nc.vector.tensor_scalar_mul(scale_col, mask_col, SCALE)
            # apply mask to V in place: vnat[j, h, jt, c] *= mask_col[j, jt]
            for h in range(HPC):
                nc.vector.tensor_tensor(
                    out=vnat[:, h],
                    in0=vnat[:, h],
                    in1=mask_col.unsqueeze(-1).broadcast_to([128, 8, 128]),
                    op=Mult,
                )

        # ================= phase 2: attention + to_out =====================
        with tc.tile_pool(name="ph2big", bufs=1) as ph2big:
            outT_sb = ph2big.tile([128, HPC, N], BF16)  # [c, h, i] 2 MB

            attn_pools = (
                tc.tile_pool(name="pexp", bufs=2),
                tc.tile_pool(name="dvp", bufs=2),
                tc.tile_pool(name="st_psum", bufs=2, space="PSUM"),
                tc.tile_pool(name="ot_psum", bufs=2, space="PSUM"),
            )
            pexp_pool, dvp, st_psum, ot_psum = (
                p.__enter__() for p in attn_pools
            )
            for h in range(HPC):
                ot = ot_psum.tile([128, N], F32)
                pexp = pexp_pool.tile([128, 8, N], BF16)

                def emit_pv(jt, ot=ot, pexp=pexp, h=h):
                    for half in range(2):
                        nc.tensor.matmul(
                            ot[:, half * 512 : (half + 1) * 512],
                            vnat[:, h, jt, :],
                            pexp[:, jt, half * 512 : (half + 1) * 512],
                            start=(jt == 0),
                            stop=(jt == 7),
                        )

                pending = None  # jt waiting for its PV emission
                for jt in range(8):
                    st = st_psum.tile([128, N], F32, tag="st")
                    for half in range(2):
                        nc.tensor.matmul(
                            st[:, half * 512 : (half + 1) * 512],
                            kT[:, h, jt * 128 : (jt + 1) * 128],
                            qT[:, h, half * 512 : (half + 1) * 512],
                            start=True,
                            stop=True,
                        )
                    nc.scalar.activation(
                        out=pexp[:, jt, :], in_=st, func=Exp,
                        scale=scale_col[:, jt : jt + 1],
                    )
                    if pending is not None:
                        emit_pv(pending)
                    pending = jt
                emit_pv(pending)

                # denominator: bf16 tree-sum over the 8 jt tiles, in place in
                # the pexp tile (safe: emitted after every PV matmul of this
                # head, so WAR deps order the overwrite behind the PE reads),
                # then partition reduce + broadcast on gpsimd (f32 accum)
                nc.vector.tensor_tensor(
                    out=pexp[:, 0:4, :], in0=pexp[:, 0:4, :],
                    in1=pexp[:, 4:8, :], op=Add,
                )
                nc.vector.tensor_tensor(
                    out=pexp[:, 0:2, :], in0=pexp[:, 0:2, :],
                    in1=pexp[:, 2:4, :], op=Add,
                )
                nc.vector.tensor_tensor(
                    out=pexp[:, 0, :], in0=pexp[:, 0, :],
                    in1=pexp[:, 1, :], op=Add,
                )
                den = dvp.tile([128, N], F32)
                nc.gpsimd.partition_all_reduce(
                    den, pexp[:, 0, :], channels=128,
                    reduce_op=bass_isa.ReduceOp.add,
                )
                rb = dvp.tile([128, N], F32)
                nc.vector.reciprocal(rb, den)
                nc.vector.tensor_tensor(
                    out=outT_sb[:, h, :], in0=ot, in1=rb, op=Mult
                )

            # ============= phase 3: to_out partial =========================
            # fo shares the st_psum slots (same shape) so Wo accumulation can
            # begin as soon as the last exp frees an ST slot.
            with tc.tile_pool(name="fout", bufs=3) as fout_pool:
                def finish_oc(oc, fo):
                    for half in range(2):
                        nc.tensor.matmul(
                            fo[:, half * 512 : (half + 1) * 512],
                            woT_sb[:, HPC - 1, oc * 128 : (oc + 1) * 128],
                            outT_sb[:, HPC - 1, half * 512 : (half + 1) * 512],
                            start=False,
                            stop=True,
                        )
                    fout = fout_pool.tile([128, N], BF16)
                    nc.vector.tensor_copy(fout, fo)
                    for sh in range(2):
                        eng = nc.sync if sh == 0 else nc.scalar
                        eng.dma_start(
                            out=outT_d[oc * 128 : (oc + 1) * 128,
                                       sh * 512 : (sh + 1) * 512],
                            in_=fout[:, sh * 512 : (sh + 1) * 512],
                        )

                pending_oc = None
                for oc in range(16):
                    fo = st_psum.tile([128, N], F32, tag="st")
                    for h in range(HPC - 1):
                        for half in range(2):
                            nc.tensor.matmul(
                                fo[:, half * 512 : (half + 1) * 512],
                                woT_sb[:, h, oc * 128 : (oc + 1) * 128],
                                outT_sb[:, h, half * 512 : (half + 1) * 512],
                                start=(h == 0),
                                stop=False,
                            )
                    if pending_oc is not None:
                        finish_oc(*pending_oc)
                    pending_oc = (oc, fo)
                finish_oc(*pending_oc)

            for p in reversed(attn_pools):
                p.__exit__(None, None, None)


_CACHE = {}


def _get_module():
    if "nc" in _CACHE:
        return _CACHE["nc"]
    nc = bacc.Bacc("TRN2", target_bir_lowering=False, debug=False, num_devices=8)
    xTd = nc.dram_tensor("xT", (D, N), F32R, kind="ExternalInput").ap()
    wq_d = nc.dram_tensor("wqT", (C, C), F32, kind="ExternalInput").ap()
    wk_d = nc.dram_tensor("wkT", (C, C), F32, kind="ExternalInput").ap()
    wv_d = nc.dram_tensor("wvT", (C, C), F32, kind="ExternalInput").ap()
    bq_d = nc.dram_tensor("bq", (C, 1), F32, kind="ExternalInput").ap()
    bk_d = nc.dram_tensor("bk", (C, 1), F32, kind="ExternalInput").ap()
    bv_d = nc.dram_tensor("bv", (C, 1), F32, kind="ExternalInput").ap()
    wtc_d = nc.dram_tensor("wtc", (C, 1), F32, kind="ExternalInput").ap()
    wo_d = nc.dram_tensor("woT", (HPC * C, D), BF16, kind="ExternalInput").ap()
    outT_d = nc.dram_tensor("outT", (D, N), BF16, kind="ExternalOutput").ap()

    with tile.TileContext(nc) as tc:
        _body(tc, xTd, wq_d, wk_d, wv_d, bq_d, bk_d, bv_d, wtc_d, wo_d, outT_d)
    nc.compile()
    _CACHE["nc"] = nc
    return nc


def make_in_maps(x, Wq, bq, Wk, bk, Wv, bv, Wl, bl, Wo, bo):
    x = np.asarray(x, np.float32)
    Wq = np.asarray(Wq, np.float32)
    Wk = np.asarray(Wk, np.float32)
    Wv = np.asarray(Wv, np.float32)
    Wl = np.asarray(Wl, np.float32)
    Wo = np.asarray(Wo, np.float32)
    we = (Wl[0] @ Wq) / float(NCHUNK)  # (128,) logits weight per chunk
    common = {
        "wqT": np.ascontiguousarray(Wq.T),
        "wkT": np.ascontiguousarray(Wk.T),
        "wvT": np.ascontiguousarray(Wv.T),
        "bq": np.asarray(bq, np.float32).reshape(C, 1),
        "bk": np.asarray(bk, np.float32).reshape(C, 1),
        "bv": np.asarray(bv, np.float32).reshape(C, 1),
        "wtc": we.astype(np.float32).reshape(C, 1),
    }
    woT = np.ascontiguousarray(Wo.T)  # (d, o)
    woT_half = [
        woT[0:1024, :].astype(ml_dtypes.bfloat16),
        woT[1024:2048, :].astype(ml_dtypes.bfloat16),
    ]
    in_maps = []
    for core in range(8):
        b, g = divmod(core, 2)
        xbT = x[b].T  # (d, n)
        xcore = np.ascontiguousarray(
            xbT if g == 0 else np.concatenate([xbT[1024:], xbT[:1024]], axis=0)
        )
        in_maps.append({"xT": xcore, "woT": woT_half[g], **common})
    return in_maps


def run_spmd(in_maps, trace=False, **kw):
    nc = _get_module()
    return bass_utils.run_bass_kernel_spmd(
        nc, in_maps, core_ids=list(range(8)), trace=trace, **kw
    )


def gather(results, bo):
    bo = np.asarray(bo, np.float32)
    out = np.empty((B, N, D), np.float32)
    for b in range(B):
        p0 = results[2 * b]["outT"].astype(np.float32).T
        p1 = results[2 * b + 1]["outT"].astype(np.float32).T
        out[b] = p0 + p1 + bo
    return out


def kernel(x, Wq, bq, Wk, bk, Wv, bv, Wl, bl, Wo, bo, stage=None, **_unused):
    in_maps = make_in_maps(x, Wq, bq, Wk, bk, Wv, bv, Wl, bl, Wo, bo)
    try:
        res = run_spmd(in_maps)
    except Exception:
        # transient device/runtime hiccup: retry once after a short pause
        import time as _time

        _time.sleep(2.0)
        res = run_spmd(in_maps)
    return gather(res.results, bo)


# revision 11
# speedup vs baseline: 1.1270x; 1.1116x over previous
"""Trainium2 Bass kernel for nn_Attention_54614804136573 (topk_masking).

Sharding: 8 cores = 4 batches x 2 head-groups (8 heads each). Each core gets
its batch's x pre-transposed on host to [d, n] (rows rotated so its own 8
head-chunks come first), computes the token-importance mask redundantly,
runs its 8 heads of attention, and produces a partial to_out product for all
2048 output channels. The host sums the two partials per batch and adds bo.

Key device-side structure (v2):
  - xT chunks are DMA'd directly (host pre-transposed), own chunks first on
    two queues; Q/K/V projections + logits consume each chunk as it lands.
  - softmax denominator is computed off the PE: pairwise bf16 tree-adds of
    the exp tiles on DVE, then a gpsimd partition_all_reduce (f32 accum,
    result broadcast to all partitions), reciprocal + fused normalize on DVE.
  - V is copied unmasked into its natural layout early; the token mask is
    applied with one in-place DVE multiply once available.
  - to_out accumulates per output-chunk over this core's 8 head-chunks with
    h-outer loop order so consecutive matmuls share stationary weights.
"""

import sys

sys.path.insert(0, "/opt/trn_rl_repo")

import numpy as np
import ml_dtypes

import concourse.mybir as mybir
import concourse.tile as tile
from concourse import bacc, bass_utils
from concourse.masks import make_identity
from concourse.tile import add_dep_helper

B = 4
N = 1024
C = 128
D = 2048
NCHUNK = 16  # d-chunks of 128 (= patch positions = heads)
HPC = 8  # heads per core
MASK_NUM = 25
SCALE = 64.0 ** -0.5  # 0.125

F32 = mybir.dt.float32
F32R = mybir.dt.float32r
BF16 = mybir.dt.bfloat16
U32 = mybir.dt.uint32
Exp = mybir.ActivationFunctionType.Exp
Ident = mybir.ActivationFunctionType.Identity
Copy = mybir.ActivationFunctionType.Copy
NEG_BIG = -1e30
Add = mybir.AluOpType.add
Mult = mybir.AluOpType.mult


def _body(tc, xTd, wq_d, wk_d, wv_d, bq_d, bk_d, bv_d, wtc_d, wo_d, outT_d):
    nc = tc.nc
    import concourse.bass_isa as bass_isa

    mscr = nc.dram_tensor("mscr", (N,), F32, kind="Internal").ap()

    with (
        tc.tile_pool(name="consts", bufs=1) as consts,
        tc.tile_pool(name="persist", bufs=1) as persist,
    ):
        # ---- constants ----
        identb = consts.tile([128, 128], BF16)
        make_identity(nc, identb)

        # ---- persistent activations ----
        qT = persist.tile([128, HPC, N], F32R)  # [c', h, n] 4 MB
        kT = persist.tile([128, HPC, N], F32R)  # 4 MB
        vnat = persist.tile([128, HPC, 8, C], BF16)  # [j, h, jt, c] 2 MB
        mask_col = persist.tile([128, 8], F32)
        scale_col = persist.tile([128, 8], F32)
        woT_sb = persist.tile([128, HPC, D], BF16)  # [d, h-chunk, o] 4 MB

        # ================= phase 1: stream xT chunks, QKV, logits, mask ====
        with (
            tc.tile_pool(name="ph1_vt", bufs=2) as ph1_vt,
            tc.tile_pool(name="ph1big", bufs=1) as ph1big,
            tc.tile_pool(name="mrows", bufs=1) as mrows,
            tc.tile_pool(name="tp_psum", bufs=2, space="PSUM") as tp_psum,
            tc.tile_pool(name="mm_psum", bufs=2, space="PSUM") as mm_psum,
            tc.tile_pool(name="lg_psum", bufs=1, space="PSUM") as lg_psum,
        ):
            xT = ph1big.tile([128, NCHUNK, N], BF16)  # [c, k, n] 4 MB

            # small weights first on the sync queue so projections can start
            # the moment chunk 0 lands
            wq_sb = consts.tile([C, C], BF16)
            nc.sync.dma_start(out=wq_sb, in_=wq_d)
            wk_sb = consts.tile([C, C], BF16)
            nc.sync.dma_start(out=wk_sb, in_=wk_d)
            wv_sb = consts.tile([C, C], BF16)
            nc.sync.dma_start(out=wv_sb, in_=wv_d)
            bq_sb = consts.tile([C, 1], F32)
            nc.sync.dma_start(out=bq_sb, in_=bq_d)
            bk_sb = consts.tile([C, 1], F32)
            nc.sync.dma_start(out=bk_sb, in_=bk_d)
            bv_sb = consts.tile([C, 1], F32)
            nc.sync.dma_start(out=bv_sb, in_=bv_d)
            wtc_sb = consts.tile([C, 1], BF16)
            nc.sync.dma_start(out=wtc_sb, in_=wtc_d)

            # own chunks 0..7 on sync; partner chunks (mask logits only) and
            # Wo on the gpsimd queue, keeping the Act sequencer DMA-free
            for k in range(HPC):
                nc.sync.dma_start(
                    out=xT[:, k, :], in_=xTd[k * 128 : (k + 1) * 128, :]
                )
            for k in range(HPC, NCHUNK):
                nc.gpsimd.dma_start(
                    out=xT[:, k, :], in_=xTd[k * 128 : (k + 1) * 128, :]
                )
            for h in range(HPC):
                nc.gpsimd.dma_start(
                    out=woT_sb[:, h, :], in_=wo_d[h * 128 : (h + 1) * 128, :]
                )

            lg = lg_psum.tile([1, N], F32)

            # per own chunk: Q/K/V projections + logits contribution
            for h in range(HPC):
                for w_sb, b_sb, dstT in ((wq_sb, bq_sb, qT), (wk_sb, bk_sb, kT)):
                    pp = mm_psum.tile([128, N], F32)
                    for half in range(2):
                        nc.tensor.matmul(
                            pp[:, half * 512 : (half + 1) * 512],
                            w_sb,
                            xT[:, h, half * 512 : (half + 1) * 512],
                            start=True,
                            stop=True,
                        )
                    nc.scalar.activation(
                        out=dstT[:, h, :], in_=pp, func=Ident, bias=b_sb
                    )
                # V: project, add bias, transpose to natural [j, c] layout
                # (unmasked for now; mask applied in-place later)
                vT_h = ph1_vt.tile([128, N], BF16)
                pp = mm_psum.tile([128, N], F32)
                for half in range(2):
                    nc.tensor.matmul(
                        pp[:, half * 512 : (half + 1) * 512],
                        wv_sb,
                        xT[:, h, half * 512 : (half + 1) * 512],
                        start=True,
                        stop=True,
                    )
                nc.scalar.activation(out=vT_h, in_=pp, func=Ident, bias=bv_sb)
                for jtg in range(2):
                    pv4 = tp_psum.tile([128, 4, 128], BF16, tag="pt4")
                    for dj in range(4):
                        jt = jtg * 4 + dj
                        nc.tensor.transpose(
                            pv4[:, dj, :], vT_h[:, jt * 128 : (jt + 1) * 128],
                            identb,
                        )
                    nc.scalar.activation(
                        out=vnat[:, h, jtg * 4 : (jtg + 1) * 4, :],
                        in_=pv4, func=Copy,
                    )
                # logits contribution for this chunk
                for half in range(2):
                    nc.tensor.matmul(
                        lg[:, half * 512 : (half + 1) * 512],
                        wtc_sb,
                        xT[:, h, half * 512 : (half + 1) * 512],
                        start=(h == 0),
                        stop=False,
                    )
            # partner chunks: logits only
            for k in range(HPC, NCHUNK):
                for half in range(2):
                    nc.tensor.matmul(
                        lg[:, half * 512 : (half + 1) * 512],
                        wtc_sb,
                        xT[:, k, half * 512 : (half + 1) * 512],
                        start=False,
                        stop=(k == NCHUNK - 1),
                    )

            # ---- mask: softmax over tokens + snap all but 25 smallest to 1
            smrow = mrows.tile([1, N], F32)
            ssum = mrows.tile([1, 1], F32)
            nc.scalar.activation(out=smrow, in_=lg, func=Exp, accum_out=ssum)
            srecip = mrows.tile([1, 1], F32)
            nc.vector.reciprocal(srecip, ssum)
            nc.vector.tensor_scalar_mul(smrow, smrow, srecip)
            negrow = mrows.tile([1, N], F32)
            nc.vector.tensor_scalar_mul(negrow, lg, -1.0)
            scratch = mrows.tile([1, N], F32)
            nc.vector.tensor_copy(scratch, negrow)
            m8 = mrows.tile([1, 8], F32)
            for _ in range(3):
                nc.vector.max(out=m8, in_=scratch)
                nc.vector.match_replace(
                    out=scratch, in_to_replace=m8, in_values=scratch,
                    imm_value=NEG_BIG,
                )
            nc.vector.max(out=m8, in_=scratch)  # m8[0,0] = 25th largest of -L
            sel = mrows.tile([1, N], U32)
            nc.vector.tensor_scalar(
                sel, negrow, m8[:, 0:1], None, op0=mybir.AluOpType.is_lt
            )
            onesrow = mrows.tile([1, N], F32)
            nc.vector.memset(onesrow, 1.0)
            nc.vector.copy_predicated(smrow, sel, onesrow)
            # row [1, 1024] -> column-major [128, 8] (partition = token % 128)
            w_inst = nc.sync.dma_start(out=mscr, in_=smrow)
            r_inst = nc.sync.dma_start(
                out=mask_col, in_=mscr.rearrange("(t j) -> j t", j=128)
            )
            add_dep_helper(r_inst.ins, w_inst.ins, sync=True, reason="mask RAW via dram")
            nc.vector.tensor_scalar_mul(scale_col, mask_col, SCALE)
            # apply mask to V in place: vnat[j, h, jt, c] *= mask_col[j, jt]
            for h in range(HPC):
                nc.vector.tensor_tensor(
                    out=vnat[:, h],
                    in0=vnat[:, h],
                    in1=mask_col.unsqueeze(-1).broadcast_to([128, 8, 128]),
                    op=Mult,
                )

        # ================= phase 2: attention + to_out =====================
        with tc.tile_pool(name="ph2big", bufs=1) as ph2big:
            outT_sb = ph2big.tile([128, HPC, N], BF16)  # [c, h, i] 2 MB

            attn_pools = (
                tc.tile_pool(name="pexp", bufs=2),
                tc.tile_pool(name="dvp", bufs=2),
                tc.tile_pool(name="st_psum", bufs=2, space="PSUM"),
                tc.tile_pool(name="ot_psum", bufs=2, space="PSUM"),
            )
            pexp_pool, dvp, st_psum, ot_psum = (
                p.__enter__() for p in attn_pools
            )
            for h in range(HPC):
                ot = ot_psum.tile([128, N], F32)
                pexp = pexp_pool.tile([128, 8, N], BF16)

                def emit_pv(jt, ot=ot, pexp=pexp, h=h):
                    for half in range(2):
                        nc.tensor.matmul(
                            ot[:, half * 512 : (half + 1) * 512],
                            vnat[:, h, jt, :],
                            pexp[:, jt, half * 512 : (half + 1) * 512],
                            start=(jt == 0),
                            stop=(jt == 7),
                        )

                pending = None  # jt waiting for its PV emission
                for jt in range(8):
                    st = st_psum.tile([128, N], F32, tag="st")
                    for half in range(2):
                        nc.tensor.matmul(
                            st[:, half * 512 : (half + 1) * 512],
                            kT[:, h, jt * 128 : (jt + 1) * 128],
                            qT[:, h, half * 512 : (half + 1) * 512],
                            start=True,
                            stop=True,
                        )
                    nc.scalar.activation(
                        out=pexp[:, jt, :], in_=st, func=Exp,
                        scale=scale_col[:, jt : jt + 1],
                    )
                    if pending is not None:
                        emit_pv(pending)
                    pending = jt
                emit_pv(pending)

                # denominator: bf16 tree-sum over the 8 jt tiles, in place in
                # the pexp tile (safe: emitted after every PV matmul of this
                # head, so WAR deps order the overwrite behind the PE reads),
                # then partition reduce + broadcast on gpsimd (f32 accum)
                nc.vector.tensor_tensor(
                    out=pexp[:, 0:4, :], in0=pexp[:, 0:4, :],
                    in1=pexp[:, 4:8, :], op=Add,
                )
                nc.vector.tensor_tensor(
                    out=pexp[:, 0:2, :], in0=pexp[:, 0:2, :],
                    in1=pexp[:, 2:4, :], op=Add,
                )
                nc.vector.tensor_tensor(
                    out=pexp[:, 0, :], in0=pexp[:, 0, :],
                    in1=pexp[:, 1, :], op=Add,
                )
                den = dvp.tile([128, N], F32)
                nc.gpsimd.partition_all_reduce(
                    den, pexp[:, 0, :], channels=128,
                    reduce_op=bass_isa.ReduceOp.add,
                )
                rb = dvp.tile([128, N], F32)
                nc.vector.reciprocal(rb, den)
                nc.vector.tensor_tensor(
                    out=outT_sb[:, h, :], in0=ot, in1=rb, op=Mult
                )

            # ============= phase 3: to_out partial =========================
            # fo shares the st_psum slots (same shape) so Wo accumulation can
            # begin as soon as the last exp frees an ST slot.
            with tc.tile_pool(name="fout", bufs=3) as fout_pool:
                def finish_oc(oc, fo):
                    for half in range(2):
                        nc.tensor.matmul(
                            fo[:, half * 512 : (half + 1) * 512],
                            woT_sb[:, HPC - 1, oc * 128 : (oc + 1) * 128],
                            outT_sb[:, HPC - 1, half * 512 : (half + 1) * 512],
                            start=False,
                            stop=True,
                        )
                    fout = fout_pool.tile([128, N], BF16)
                    nc.vector.tensor_copy(fout, fo)
                    for sh in range(2):
                        eng = nc.sync if sh == 0 else nc.scalar
                        eng.dma_start(
                            out=outT_d[oc * 128 : (oc + 1) * 128,
                                       sh * 512 : (sh + 1) * 512],
                            in_=fout[:, sh * 512 : (sh + 1) * 512],
                        )

                pending_oc = None
                for oc in range(16):
                    fo = st_psum.tile([128, N], F32, tag="st")
                    for h in range(HPC - 1):
                        for half in range(2):
                            nc.tensor.matmul(
                                fo[:, half * 512 : (half + 1) * 512],
                                woT_sb[:, h, oc * 128 : (oc + 1) * 128],
                                outT_sb[:, h, half * 512 : (half + 1) * 512],
                                start=(h == 0),
                                stop=False,
                            )
                    if pending_oc is not None:
                        finish_oc(*pending_oc)
                    pending_oc = (oc, fo)
                finish_oc(*pending_oc)

            for p in reversed(attn_pools):
                p.__exit__(None, None, None)


_CACHE = {}


def _get_module():
    if "nc" in _CACHE:
        return _CACHE["nc"]
    nc = bacc.Bacc("TRN2", target_bir_lowering=False, debug=False, num_devices=8)
    xTd = nc.dram_tensor("xT", (D, N), BF16, kind="ExternalInput").ap()
    wq_d = nc.dram_tensor("wqT", (C, C), BF16, kind="ExternalInput").ap()
    wk_d = nc.dram_tensor("wkT", (C, C), BF16, kind="ExternalInput").ap()
    wv_d = nc.dram_tensor("wvT", (C, C), BF16, kind="ExternalInput").ap()
    bq_d = nc.dram_tensor("bq", (C, 1), F32, kind="ExternalInput").ap()
    bk_d = nc.dram_tensor("bk", (C, 1), F32, kind="ExternalInput").ap()
    bv_d = nc.dram_tensor("bv", (C, 1), F32, kind="ExternalInput").ap()
    wtc_d = nc.dram_tensor("wtc", (C, 1), BF16, kind="ExternalInput").ap()
    wo_d = nc.dram_tensor("woT", (HPC * C, D), BF16, kind="ExternalInput").ap()
    outT_d = nc.dram_tensor("outT", (D, N), BF16, kind="ExternalOutput").ap()

    with tile.TileContext(nc) as tc:
        _body(tc, xTd, wq_d, wk_d, wv_d, bq_d, bk_d, bv_d, wtc_d, wo_d, outT_d)
    nc.compile()
    _CACHE["nc"] = nc
    return nc


def make_in_maps(x, Wq, bq, Wk, bk, Wv, bv, Wl, bl, Wo, bo):
    x = np.asarray(x, np.float32)
    Wq = np.asarray(Wq, np.float32)
    Wk = np.asarray(Wk, np.float32)
    Wv = np.asarray(Wv, np.float32)
    Wl = np.asarray(Wl, np.float32)
    Wo = np.asarray(Wo, np.float32)
    we = (Wl[0] @ Wq) / float(NCHUNK)  # (128,) logits weight per chunk
    bf = ml_dtypes.bfloat16
    common = {
        "wqT": np.ascontiguousarray(Wq.T).astype(bf),
        "wkT": np.ascontiguousarray(Wk.T).astype(bf),
        "wvT": np.ascontiguousarray(Wv.T).astype(bf),
        "bq": np.asarray(bq, np.float32).reshape(C, 1),
        "bk": np.asarray(bk, np.float32).reshape(C, 1),
        "bv": np.asarray(bv, np.float32).reshape(C, 1),
        "wtc": we.reshape(C, 1).astype(bf),
    }
    woT = np.ascontiguousarray(Wo.T)  # (d, o)
    woT_half = [
        woT[0:1024, :].astype(ml_dtypes.bfloat16),
        woT[1024:2048, :].astype(ml_dtypes.bfloat16),
    ]
    in_maps = []
    xT_halves = [
        np.ascontiguousarray(x[b].T).astype(bf) for b in range(B)
    ]
    for core in range(8):
        b, g = divmod(core, 2)
        xbT = xT_halves[b]
        xcore = xbT if g == 0 else np.ascontiguousarray(
            np.concatenate([xbT[1024:], xbT[:1024]], axis=0)
        )
        in_maps.append({"xT": xcore, "woT": woT_half[g], **common})
    return in_maps


def run_spmd(in_maps, trace=False, **kw):
    nc = _get_module()
    return bass_utils.run_bass_kernel_spmd(
        nc, in_maps, core_ids=list(range(8)), trace=trace, **kw
    )


def gather(results, bo):
    bo = np.asarray(bo, np.float32)
    out = np.empty((B, N, D), np.float32)
    for b in range(B):
        p0 = results[2 * b]["outT"].astype(np.float32).T
        p1 = results[2 * b + 1]["outT"].astype(np.float32).T
        out[b] = p0 + p1 + bo
    return out


def kernel(x, Wq, bq, Wk, bk, Wv, bv, Wl, bl, Wo, bo, stage=None, **_unused):
    in_maps = make_in_maps(x, Wq, bq, Wk, bk, Wv, bv, Wl, bl, Wo, bo)
    try:
        res = run_spmd(in_maps)
    except Exception:
        # transient device/runtime hiccup: retry once after a short pause
        import time as _time

        _time.sleep(2.0)
        res = run_spmd(in_maps)
    return gather(res.results, bo)


# revision 16
# speedup vs baseline: 1.1299x; 1.0025x over previous
"""Trainium2 Bass kernel for nn_Attention_54614804136573 (topk_masking).

Sharding: 8 cores = 4 batches x 2 head-groups (8 heads each). Each core gets
its batch's x pre-transposed on host to bf16 [d, n] (rows rotated so its own
8 head-chunks come first), computes the token-importance mask redundantly,
runs its 8 heads of attention, and produces a partial to_out product for all
2048 output channels. The host sums the two partials per batch and adds bo.

Device-side structure (v4):
  - xT chunks are DMA'd directly in bf16; Q/K/V projections + logits consume
    each chunk as it lands. V is transposed to its natural layout with DMA
    transposes (no PE/PSUM involvement) and masked in place on gpsimd.
  - the token mask is binary: the bottom-25 softmax values are ~1e-3 and are
    snapped to 0 (their exp contribution becomes exactly 1 via a 0 scale,
    their V rows 0), which is well inside the error budget.
  - softmax denominator off the PE: pairwise bf16 tree-adds of the exp tiles
    on DVE (in place in the pexp tile), then a gpsimd partition_all_reduce
    (f32 accum, broadcast), reciprocal + fused normalize on DVE.
  - to_out accumulates per output-chunk over head-chunks 1..7 (h-outer so
    consecutive matmuls share stationary weights) and finishes with head 0,
    whose outT is ready first.
"""

import sys

sys.path.insert(0, "/opt/trn_rl_repo")

import numpy as np
import ml_dtypes

import concourse.mybir as mybir
import concourse.tile as tile
from concourse import bacc, bass_utils
from concourse import bass_isa
from concourse.tile import add_dep_helper

B = 4
N = 1024
C = 128
D = 2048
NCHUNK = 16  # d-chunks of 128 (= patch positions = heads)
HPC = 8  # heads per core
MASK_NUM = 25
SCALE = 64.0 ** -0.5  # 0.125

F32 = mybir.dt.float32
BF16 = mybir.dt.bfloat16
U32 = mybir.dt.uint32
Exp = mybir.ActivationFunctionType.Exp
Ident = mybir.ActivationFunctionType.Identity
Copy = mybir.ActivationFunctionType.Copy
NEG_BIG = -1e30
Add = mybir.AluOpType.add
Mult = mybir.AluOpType.mult


def _body(tc, xTd, wq_d, wk_d, wv_d, bq_d, bk_d, bv_d, wtc_d, wo_d, outT_d):
    nc = tc.nc
    mscr = nc.dram_tensor("mscr", (N,), F32, kind="Internal").ap()

    with (
        tc.tile_pool(name="consts", bufs=1) as consts,
        tc.tile_pool(name="persist", bufs=1) as persist,
    ):
        # ---- persistent activations ----
        qT = persist.tile([128, HPC, N], BF16)  # [c', h, n] 2 MB
        kT = persist.tile([128, HPC, N], BF16)  # 2 MB
        vnat = persist.tile([128, HPC, 8, C], BF16)  # [j, h, jt, c] 2 MB
        mask_col = persist.tile([128, 8], F32)
        scale_col = persist.tile([128, 8], F32)
        woT_sb = persist.tile([128, HPC, D], BF16)  # [d, h-chunk, o] 4 MB

        # ================= phase 1: stream xT chunks, QKV, logits, mask ====
        with (
            tc.tile_pool(name="ph1_vt", bufs=2) as ph1_vt,
            tc.tile_pool(name="ph1big", bufs=1) as ph1big,
            tc.tile_pool(name="mrows", bufs=1) as mrows,
            tc.tile_pool(name="mm_psum", bufs=2, space="PSUM") as mm_psum,
            tc.tile_pool(name="lg_psum", bufs=1, space="PSUM") as lg_psum,
        ):
            xT = ph1big.tile([128, NCHUNK, N], BF16)  # [c, k, n] 4 MB

            # small weights first on the sync queue so projections can start
            # the moment chunk 0 lands
            wq_sb = consts.tile([C, C], BF16)
            nc.sync.dma_start(out=wq_sb, in_=wq_d)
            wk_sb = consts.tile([C, C], BF16)
            nc.sync.dma_start(out=wk_sb, in_=wk_d)
            wv_sb = consts.tile([C, C], BF16)
            nc.sync.dma_start(out=wv_sb, in_=wv_d)
            bq_sb = consts.tile([C, 1], F32)
            nc.sync.dma_start(out=bq_sb, in_=bq_d)
            bk_sb = consts.tile([C, 1], F32)
            nc.sync.dma_start(out=bk_sb, in_=bk_d)
            bv_sb = consts.tile([C, 1], F32)
            nc.sync.dma_start(out=bv_sb, in_=bv_d)
            wtc_sb = consts.tile([C, 1], BF16)
            nc.sync.dma_start(out=wtc_sb, in_=wtc_d)

            # own chunks 0..7 on sync; partner chunks (mask logits only) and
            # Wo on the gpsimd queue, keeping the Act sequencer DMA-free
            for k in range(HPC):
                nc.sync.dma_start(
                    out=xT[:, k, :], in_=xTd[k * 128 : (k + 1) * 128, :]
                )
            for k in range(HPC, NCHUNK):
                nc.gpsimd.dma_start(
                    out=xT[:, k, :], in_=xTd[k * 128 : (k + 1) * 128, :]
                )
            for h in range(HPC):
                nc.gpsimd.dma_start(
                    out=woT_sb[:, h, :], in_=wo_d[h * 128 : (h + 1) * 128, :]
                )

            # mask-row scratch, zero/one rows prepared off the critical path
            onesrow = mrows.tile([1, N], F32)
            nc.vector.memset(onesrow, 1.0)
            maskrow = mrows.tile([1, N], F32)
            nc.vector.memset(maskrow, 0.0)

            lg = lg_psum.tile([1, N], F32)

            # per own chunk: Q/K/V projections + logits contributions
            # (own chunk h, partner chunk 8+h as its DMA lands)
            for h in range(HPC):
                for w_sb, b_sb, dstT in ((wq_sb, bq_sb, qT), (wk_sb, bk_sb, kT)):
                    pp = mm_psum.tile([128, N], F32)
                    for half in range(2):
                        nc.tensor.matmul(
                            pp[:, half * 512 : (half + 1) * 512],
                            w_sb,
                            xT[:, h, half * 512 : (half + 1) * 512],
                            start=True,
                            stop=True,
                        )
                    nc.scalar.activation(
                        out=dstT[:, h, :], in_=pp, func=Ident, bias=b_sb
                    )
                # V: project, add bias, DMA-transpose to natural [j, c]
                # layout (unmasked; masked in place on gpsimd once the mask
                # is known)
                vT_h = ph1_vt.tile([128, N], BF16)
                pp = mm_psum.tile([128, N], F32)
                for half in range(2):
                    nc.tensor.matmul(
                        pp[:, half * 512 : (half + 1) * 512],
                        wv_sb,
                        xT[:, h, half * 512 : (half + 1) * 512],
                        start=True,
                        stop=True,
                    )
                nc.scalar.activation(out=vT_h, in_=pp, func=Ident, bias=bv_sb)
                nc.sync.dma_start_transpose(out=vnat[:, h], in_=vT_h)
                # logits contributions
                for half in range(2):
                    nc.tensor.matmul(
                        lg[:, half * 512 : (half + 1) * 512],
                        wtc_sb,
                        xT[:, h, half * 512 : (half + 1) * 512],
                        start=(h == 0),
                        stop=False,
                    )
                k = HPC + h
                for half in range(2):
                    nc.tensor.matmul(
                        lg[:, half * 512 : (half + 1) * 512],
                        wtc_sb,
                        xT[:, k, half * 512 : (half + 1) * 512],
                        start=False,
                        stop=(k == NCHUNK - 1),
                    )

            # ---- binary mask: 1 for tokens above the 25th-smallest logit,
            # 0 for the bottom 25 (their softmax values are ~1e-3; dropping
            # them costs ~1e-3 relative error)
            scratch = mrows.tile([1, N], F32)
            nc.vector.tensor_scalar_mul(scratch, lg, -1.0)
            m8 = mrows.tile([1, 8], F32)
            for _ in range(3):
                nc.vector.max(out=m8, in_=scratch)
                nc.vector.match_replace(
                    out=scratch, in_to_replace=m8, in_values=scratch,
                    imm_value=NEG_BIG,
                )
            nc.vector.max(out=m8, in_=scratch)  # m8[0,0] = 25th largest of -L
            m8neg = mrows.tile([1, 1], F32)
            nc.vector.tensor_scalar_mul(m8neg, m8[:, 0:1], -1.0)
            sel = mrows.tile([1, N], U32)
            nc.vector.tensor_scalar(
                sel, lg, m8neg, None, op0=mybir.AluOpType.is_gt
            )
            nc.vector.copy_predicated(maskrow, sel, onesrow)
            # row [1, 1024] -> column-major [128, 8] (partition = token % 128)
            # via a DRAM roundtrip
            w_inst = nc.scalar.dma_start(out=mscr, in_=maskrow)
            r_inst = nc.scalar.dma_start(
                out=mask_col, in_=mscr.rearrange("(t j) -> j t", j=128)
            )
            add_dep_helper(r_inst.ins, w_inst.ins, sync=True, reason="mask RAW via dram")
            nc.vector.tensor_scalar_mul(scale_col, mask_col, SCALE)
            # apply mask to V in place on gpsimd (Pool is otherwise idle)
            for h in range(HPC):
                nc.gpsimd.tensor_tensor(
                    out=vnat[:, h],
                    in0=vnat[:, h],
                    in1=mask_col.unsqueeze(-1).broadcast_to([128, 8, 128]),
                    op=Mult,
                )

        # ================= phase 2: attention + to_out =====================
        with tc.tile_pool(name="ph2big", bufs=1) as ph2big:
            outT_sb = ph2big.tile([128, HPC, N], BF16)  # [c, h, i] 2 MB

            attn_pools = (
                tc.tile_pool(name="pexp", bufs=2),
                tc.tile_pool(name="dvp", bufs=2),
                tc.tile_pool(name="st_psum", bufs=2, space="PSUM"),
                tc.tile_pool(name="ot_psum", bufs=2, space="PSUM"),
            )
            pexp_pool, dvp, st_psum, ot_psum = (
                p.__enter__() for p in attn_pools
            )
            for h in range(HPC):
                ot = ot_psum.tile([128, N], F32)
                pexp = pexp_pool.tile([128, 8, N], BF16)

                def emit_pv(jt, ot=ot, pexp=pexp, h=h):
                    for half in range(2):
                        nc.tensor.matmul(
                            ot[:, half * 512 : (half + 1) * 512],
                            vnat[:, h, jt, :],
                            pexp[:, jt, half * 512 : (half + 1) * 512],
                            start=(jt == 0),
                            stop=(jt == 7),
                        )

                pending = None  # jt waiting for its PV emission
                for jt in range(8):
                    st = st_psum.tile([128, N], F32, tag="st")
                    for half in range(2):
                        nc.tensor.matmul(
                            st[:, half * 512 : (half + 1) * 512],
                            kT[:, h, jt * 128 : (jt + 1) * 128],
                            qT[:, h, half * 512 : (half + 1) * 512],
                            start=True,
                            stop=True,
                        )
                    nc.scalar.activation(
                        out=pexp[:, jt, :], in_=st, func=Exp,
                        scale=scale_col[:, jt : jt + 1],
                    )
                    if pending is not None:
                        emit_pv(pending)
                    pending = jt
                emit_pv(pending)

                # denominator: bf16 tree-sum over the 8 jt tiles, in place in
                # the pexp tile (safe: emitted after every PV matmul of this
                # head, so WAR deps order the overwrite behind the PE reads),
                # then partition reduce + broadcast on gpsimd (f32 accum)
                nc.vector.tensor_tensor(
                    out=pexp[:, 0:4, :], in0=pexp[:, 0:4, :],
                    in1=pexp[:, 4:8, :], op=Add,
                )
                nc.vector.tensor_tensor(
                    out=pexp[:, 0:2, :], in0=pexp[:, 0:2, :],
                    in1=pexp[:, 2:4, :], op=Add,
                )
                nc.vector.tensor_tensor(
                    out=pexp[:, 0, :], in0=pexp[:, 0, :],
                    in1=pexp[:, 1, :], op=Add,
                )
                den = dvp.tile([128, N], F32)
                nc.gpsimd.partition_all_reduce(
                    den, pexp[:, 0, :], channels=128,
                    reduce_op=bass_isa.ReduceOp.add,
                )
                rb = dvp.tile([128, N], F32)
                nc.vector.reciprocal(rb, den)
                nc.vector.tensor_tensor(
                    out=outT_sb[:, h, :], in0=ot, in1=rb, op=Mult
                )

            # ============= phase 3: to_out partial =========================
            # fo shares the st_psum slots (same shape) so Wo accumulation can
            # begin as soon as the last exp frees an ST slot. Heads 1..7 are
            # accumulated in the per-oc loop; head 0 (whose outT is ready
            # first) finishes each oc one iteration later.
            with tc.tile_pool(name="fout", bufs=3) as fout_pool:
                def finish_oc(oc, fo):
                    for half in range(2):
                        nc.tensor.matmul(
                            fo[:, half * 512 : (half + 1) * 512],
                            woT_sb[:, 0, oc * 128 : (oc + 1) * 128],
                            outT_sb[:, 0, half * 512 : (half + 1) * 512],
                            start=False,
                            stop=True,
                        )
                    fout = fout_pool.tile([128, N], BF16)
                    nc.scalar.activation(out=fout, in_=fo, func=Copy)
                    for sh in range(2):
                        eng = nc.sync if sh == 0 else nc.scalar
                        eng.dma_start(
                            out=outT_d[oc * 128 : (oc + 1) * 128,
                                       sh * 512 : (sh + 1) * 512],
                            in_=fout[:, sh * 512 : (sh + 1) * 512],
                        )

                pending_oc = None
                for oc in range(16):
                    fo = st_psum.tile([128, N], F32, tag="st")
                    for h in range(1, HPC):
                        for half in range(2):
                            nc.tensor.matmul(
                                fo[:, half * 512 : (half + 1) * 512],
                                woT_sb[:, h, oc * 128 : (oc + 1) * 128],
                                outT_sb[:, h, half * 512 : (half + 1) * 512],
                                start=(h == 1),
                                stop=False,
                            )
                    if pending_oc is not None:
                        finish_oc(*pending_oc)
                    pending_oc = (oc, fo)
                finish_oc(*pending_oc)

            for p in reversed(attn_pools):
                p.__exit__(None, None, None)


_CACHE = {}


def _get_module():
    if "nc" in _CACHE:
        return _CACHE["nc"]
    nc = bacc.Bacc("TRN2", target_bir_lowering=False, debug=False, num_devices=8)
    xTd = nc.dram_tensor("xT", (D, N), BF16, kind="ExternalInput").ap()
    wq_d = nc.dram_tensor("wqT", (C, C), BF16, kind="ExternalInput").ap()
    wk_d = nc.dram_tensor("wkT", (C, C), BF16, kind="ExternalInput").ap()
    wv_d = nc.dram_tensor("wvT", (C, C), BF16, kind="ExternalInput").ap()
    bq_d = nc.dram_tensor("bq", (C, 1), F32, kind="ExternalInput").ap()
    bk_d = nc.dram_tensor("bk", (C, 1), F32, kind="ExternalInput").ap()
    bv_d = nc.dram_tensor("bv", (C, 1), F32, kind="ExternalInput").ap()
    wtc_d = nc.dram_tensor("wtc", (C, 1), BF16, kind="ExternalInput").ap()
    wo_d = nc.dram_tensor("woT", (HPC * C, D), BF16, kind="ExternalInput").ap()
    outT_d = nc.dram_tensor("outT", (D, N), BF16, kind="ExternalOutput").ap()

    with tile.TileContext(nc) as tc:
        _body(tc, xTd, wq_d, wk_d, wv_d, bq_d, bk_d, bv_d, wtc_d, wo_d, outT_d)
    nc.compile()
    _CACHE["nc"] = nc
    return nc


def make_in_maps(x, Wq, bq, Wk, bk, Wv, bv, Wl, bl, Wo, bo):
    x = np.asarray(x, np.float32)
    Wq = np.asarray(Wq, np.float32)
    Wk = np.asarray(Wk, np.float32)
    Wv = np.asarray(Wv, np.float32)
    Wl = np.asarray(Wl, np.float32)
    Wo = np.asarray(Wo, np.float32)
    we = (Wl[0] @ Wq) / float(NCHUNK)  # (128,) logits weight per chunk
    bf = ml_dtypes.bfloat16
    common = {
        "wqT": np.ascontiguousarray(Wq.T).astype(bf),
        "wkT": np.ascontiguousarray(Wk.T).astype(bf),
        "wvT": np.ascontiguousarray(Wv.T).astype(bf),
        "bq": np.asarray(bq, np.float32).reshape(C, 1),
        "bk": np.asarray(bk, np.float32).reshape(C, 1),
        "bv": np.asarray(bv, np.float32).reshape(C, 1),
        "wtc": we.reshape(C, 1).astype(bf),
    }
    woT = np.ascontiguousarray(Wo.T)  # (d, o)
    woT_half = [
        woT[0:1024, :].astype(bf),
        woT[1024:2048, :].astype(bf),
    ]
    in_maps = []
    xT_whole = [np.ascontiguousarray(x[b].T).astype(bf) for b in range(B)]
    for core in range(8):
        b, g = divmod(core, 2)
        xbT = xT_whole[b]
        xcore = xbT if g == 0 else np.ascontiguousarray(
            np.concatenate([xbT[1024:], xbT[:1024]], axis=0)
        )
        in_maps.append({"xT": xcore, "woT": woT_half[g], **common})
    return in_maps


def run_spmd(in_maps, trace=False, **kw):
    nc = _get_module()
    return bass_utils.run_bass_kernel_spmd(
        nc, in_maps, core_ids=list(range(8)), trace=trace, **kw
    )


def gather(results, bo):
    bo = np.asarray(bo, np.float32)
    out = np.empty((B, N, D), np.float32)
    for b in range(B):
        p0 = results[2 * b]["outT"].astype(np.float32).T
        p1 = results[2 * b + 1]["outT"].astype(np.float32).T
        out[b] = p0 + p1 + bo
    return out


def kernel(x, Wq, bq, Wk, bk, Wv, bv, Wl, bl, Wo, bo, stage=None, **_unused):
    in_maps = make_in_maps(x, Wq, bq, Wk, bk, Wv, bv, Wl, bl, Wo, bo)
    try:
        res = run_spmd(in_maps)
    except Exception:
        # transient device/runtime hiccup: retry once after a short pause
        import time as _time

        _time.sleep(2.0)
        res = run_spmd(in_maps)
    return gather(res.results, bo)


# revision 19
# speedup vs baseline: 1.2092x; 1.0702x over previous
"""Trainium2 Bass kernel for nn_Attention_54614804136573 (topk_masking).

Sharding: 8 cores = 4 batches x 2 head-groups (8 heads each). Each core gets
its batch's x pre-transposed on host to bf16 [d, n] (rows rotated so its own
8 head-chunks come first), computes the token-importance mask redundantly,
runs its 8 heads of attention, and produces a partial to_out product for all
2048 output channels. The host sums the two partials per batch and adds bo.

Device-side structure (v4):
  - xT chunks are DMA'd directly in bf16; Q/K/V projections + logits consume
    each chunk as it lands. V is transposed to its natural layout with DMA
    transposes (no PE/PSUM involvement) and masked in place on gpsimd.
  - the token mask is binary: the bottom-25 softmax values are ~1e-3 and are
    snapped to 0 (their exp contribution becomes exactly 1 via a 0 scale,
    their V rows 0), which is well inside the error budget.
  - softmax denominator off the PE: pairwise bf16 tree-adds of the exp tiles
    on DVE (in place in the pexp tile), then a gpsimd partition_all_reduce
    (f32 accum, broadcast), reciprocal + fused normalize on DVE.
  - to_out accumulates per output-chunk over head-chunks 1..7 (h-outer so
    consecutive matmuls share stationary weights) and finishes with head 0,
    whose outT is ready first.
"""

import sys

sys.path.insert(0, "/opt/trn_rl_repo")

import numpy as np
import ml_dtypes

import concourse.mybir as mybir
import concourse.tile as tile
from concourse import bacc, bass_utils
from concourse import bass_isa
from concourse.tile import add_dep_helper

B = 4
N = 1024
C = 128
D = 2048
NCHUNK = 16  # d-chunks of 128 (= patch positions = heads)
HPC = 8  # heads per core
MASK_NUM = 25
SCALE = 64.0 ** -0.5  # 0.125

F32 = mybir.dt.float32
BF16 = mybir.dt.bfloat16
U32 = mybir.dt.uint32
Exp = mybir.ActivationFunctionType.Exp
Ident = mybir.ActivationFunctionType.Identity
Copy = mybir.ActivationFunctionType.Copy
NEG_BIG = -1e30
Add = mybir.AluOpType.add
Mult = mybir.AluOpType.mult


def _body(tc, xTd, wq_d, wk_d, wv_d, bq_d, bk_d, bv_d, wtc_d, wo_d, outT_d):
    nc = tc.nc
    mscr = nc.dram_tensor("mscr", (N,), F32, kind="Internal").ap()

    with (
        tc.tile_pool(name="consts", bufs=1) as consts,
        tc.tile_pool(name="persist", bufs=1) as persist,
    ):
        # ---- persistent activations ----
        qT = persist.tile([128, HPC, N], BF16)  # [c', h, n] 2 MB
        kT = persist.tile([128, HPC, N], BF16)  # 2 MB
        vnat = persist.tile([128, HPC, 8, C], BF16)  # [j, h, jt, c] 2 MB
        mask_col = persist.tile([128, 8], F32)
        scale_col = persist.tile([128, 8], F32)
        woT_sb = persist.tile([128, HPC, D], BF16)  # [d, h-chunk, o] 4 MB

        # ================= phase 1: stream xT chunks, QKV, logits, mask ====
        with (
            tc.tile_pool(name="ph1_vt", bufs=4) as ph1_vt,
            tc.tile_pool(name="ph1big", bufs=1) as ph1big,
            tc.tile_pool(name="mrows", bufs=1) as mrows,
            tc.tile_pool(name="mm_psum", bufs=2, space="PSUM") as mm_psum,
            tc.tile_pool(name="lg_psum", bufs=1, space="PSUM") as lg_psum,
        ):
            xT = ph1big.tile([128, NCHUNK, N], BF16)  # [c, k, n] 4 MB

            # small weights first on the sync queue so projections can start
            # the moment chunk 0 lands
            wq_sb = consts.tile([C, C], BF16)
            nc.sync.dma_start(out=wq_sb, in_=wq_d)
            wk_sb = consts.tile([C, C], BF16)
            nc.sync.dma_start(out=wk_sb, in_=wk_d)
            wv_sb = consts.tile([C, C], BF16)
            nc.sync.dma_start(out=wv_sb, in_=wv_d)
            bq_sb = consts.tile([C, 1], F32)
            nc.sync.dma_start(out=bq_sb, in_=bq_d)
            bk_sb = consts.tile([C, 1], F32)
            nc.sync.dma_start(out=bk_sb, in_=bk_d)
            bv_sb = consts.tile([C, 1], F32)
            nc.sync.dma_start(out=bv_sb, in_=bv_d)
            wtc_sb = consts.tile([C, 1], BF16)
            nc.sync.dma_start(out=wtc_sb, in_=wtc_d)

            # own chunks 0..7 on sync; partner chunks (mask logits only) and
            # Wo on the gpsimd queue, keeping the Act sequencer DMA-free
            for k in range(HPC):
                nc.sync.dma_start(
                    out=xT[:, k, :], in_=xTd[k * 128 : (k + 1) * 128, :]
                )
            for k in range(HPC, NCHUNK):
                nc.gpsimd.dma_start(
                    out=xT[:, k, :], in_=xTd[k * 128 : (k + 1) * 128, :]
                )

            # mask-row scratch, zero/one rows prepared off the critical path
            onesrow = mrows.tile([1, N], F32)
            nc.vector.memset(onesrow, 1.0)
            maskrow = mrows.tile([1, N], F32)
            nc.vector.memset(maskrow, 0.0)

            lg = lg_psum.tile([1, N], F32)

            # per own chunk: Q/K/V projections + logits contributions
            # (own chunk h, partner chunk 8+h as its DMA lands)
            for h in range(HPC):
                for w_sb, b_sb, dstT in ((wq_sb, bq_sb, qT), (wk_sb, bk_sb, kT)):
                    pp = mm_psum.tile([128, N], F32)
                    for half in range(2):
                        nc.tensor.matmul(
                            pp[:, half * 512 : (half + 1) * 512],
                            w_sb,
                            xT[:, h, half * 512 : (half + 1) * 512],
                            start=True,
                            stop=True,
                        )
                    nc.scalar.activation(
                        out=dstT[:, h, :], in_=pp, func=Ident, bias=b_sb
                    )
                # V: project, add bias, DMA-transpose to natural [j, c]
                # layout (unmasked; masked in place on gpsimd once the mask
                # is known)
                vT_h = ph1_vt.tile([128, N], BF16)
                pp = mm_psum.tile([128, N], F32)
                for half in range(2):
                    nc.tensor.matmul(
                        pp[:, half * 512 : (half + 1) * 512],
                        wv_sb,
                        xT[:, h, half * 512 : (half + 1) * 512],
                        start=True,
                        stop=True,
                    )
                nc.scalar.activation(out=vT_h, in_=pp, func=Ident, bias=bv_sb)
                nc.sync.dma_start_transpose(out=vnat[:, h], in_=vT_h)
                # logits contributions
                for half in range(2):
                    nc.tensor.matmul(
                        lg[:, half * 512 : (half + 1) * 512],
                        wtc_sb,
                        xT[:, h, half * 512 : (half + 1) * 512],
                        start=(h == 0),
                        stop=False,
                    )
                k = HPC + h
                for half in range(2):
                    nc.tensor.matmul(
                        lg[:, half * 512 : (half + 1) * 512],
                        wtc_sb,
                        xT[:, k, half * 512 : (half + 1) * 512],
                        start=False,
                        stop=(k == NCHUNK - 1),
                    )

            # ---- binary mask: 1 for tokens above the 25th-smallest logit,
            # 0 for the bottom 25 (their softmax values are ~1e-3; dropping
            # them costs ~1e-3 relative error)
            scratch = mrows.tile([1, N], F32)
            nc.vector.tensor_scalar_mul(scratch, lg, -1.0)
            m8 = mrows.tile([1, 8], F32)
            for _ in range(3):
                nc.vector.max(out=m8, in_=scratch)
                nc.vector.match_replace(
                    out=scratch, in_to_replace=m8, in_values=scratch,
                    imm_value=NEG_BIG,
                )
            nc.vector.max(out=m8, in_=scratch)  # m8[0,0] = 25th largest of -L
            m8neg = mrows.tile([1, 1], F32)
            nc.vector.tensor_scalar_mul(m8neg, m8[:, 0:1], -1.0)
            sel = mrows.tile([1, N], U32)
            nc.vector.tensor_scalar(
                sel, lg, m8neg, None, op0=mybir.AluOpType.is_gt
            )
            nc.vector.copy_predicated(maskrow, sel, onesrow)
            # row [1, 1024] -> column-major [128, 8] (partition = token % 128)
            # via a DRAM roundtrip
            w_inst = nc.scalar.dma_start(out=mscr, in_=maskrow)
            r_inst = nc.scalar.dma_start(
                out=mask_col, in_=mscr.rearrange("(t j) -> j t", j=128)
            )
            add_dep_helper(r_inst.ins, w_inst.ins, sync=True, reason="mask RAW via dram")
            nc.vector.tensor_scalar_mul(scale_col, mask_col, SCALE)
            # Wo (4 MB bf16): deliberately deferred behind the mask roundtrip
            # on the serial DMA path — it is only needed by to_out, and ahead
            # of the mask it would delay the whole attention phase
            for h in range(HPC):
                wo_i = nc.sync.dma_start(
                    out=woT_sb[:, h, :], in_=wo_d[h * 128 : (h + 1) * 128, :]
                )
                add_dep_helper(
                    wo_i.ins, r_inst.ins, sync=True, reason="wo after mask dma"
                )
            # apply mask to V in place on gpsimd (Pool is otherwise idle)
            for h in range(HPC):
                nc.gpsimd.tensor_tensor(
                    out=vnat[:, h],
                    in0=vnat[:, h],
                    in1=mask_col.unsqueeze(-1).broadcast_to([128, 8, 128]),
                    op=Mult,
                )

        # ================= phase 2: attention + to_out =====================
        with tc.tile_pool(name="ph2big", bufs=1) as ph2big:
            outT_sb = ph2big.tile([128, HPC, N], BF16)  # [c, h, i] 2 MB

            attn_pools = (
                tc.tile_pool(name="pexp", bufs=2),
                tc.tile_pool(name="dvp", bufs=2),
                tc.tile_pool(name="st_psum", bufs=2, space="PSUM"),
                tc.tile_pool(name="ot_psum", bufs=2, space="PSUM"),
            )
            pexp_pool, dvp, st_psum, ot_psum = (
                p.__enter__() for p in attn_pools
            )
            for h in range(HPC):
                ot = ot_psum.tile([128, N], F32)
                pexp = pexp_pool.tile([128, 8, N], BF16)

                def emit_pv(jt, ot=ot, pexp=pexp, h=h):
                    for half in range(2):
                        nc.tensor.matmul(
                            ot[:, half * 512 : (half + 1) * 512],
                            vnat[:, h, jt, :],
                            pexp[:, jt, half * 512 : (half + 1) * 512],
                            start=(jt == 0),
                            stop=(jt == 7),
                        )

                pending = None  # jt waiting for its PV emission
                for jt in range(8):
                    st = st_psum.tile([128, N], F32, tag="st")
                    for half in range(2):
                        nc.tensor.matmul(
                            st[:, half * 512 : (half + 1) * 512],
                            kT[:, h, jt * 128 : (jt + 1) * 128],
                            qT[:, h, half * 512 : (half + 1) * 512],
                            start=True,
                            stop=True,
                        )
                    nc.scalar.activation(
                        out=pexp[:, jt, :], in_=st, func=Exp,
                        scale=scale_col[:, jt : jt + 1],
                    )
                    if pending is not None:
                        emit_pv(pending)
                    pending = jt
                emit_pv(pending)

                # denominator: bf16 tree-sum over the 8 jt tiles, in place in
                # the pexp tile (safe: emitted after every PV matmul of this
                # head, so WAR deps order the overwrite behind the PE reads),
                # then partition reduce + broadcast on gpsimd (f32 accum)
                nc.vector.tensor_tensor(
                    out=pexp[:, 0:4, :], in0=pexp[:, 0:4, :],
                    in1=pexp[:, 4:8, :], op=Add,
                )
                nc.vector.tensor_tensor(
                    out=pexp[:, 0:2, :], in0=pexp[:, 0:2, :],
                    in1=pexp[:, 2:4, :], op=Add,
                )
                nc.vector.tensor_tensor(
                    out=pexp[:, 0, :], in0=pexp[:, 0, :],
                    in1=pexp[:, 1, :], op=Add,
                )
                den = dvp.tile([128, N], F32)
                nc.gpsimd.partition_all_reduce(
                    den, pexp[:, 0, :], channels=128,
                    reduce_op=bass_isa.ReduceOp.add,
                )
                rb = dvp.tile([128, N], F32)
                nc.vector.reciprocal(rb, den)
                nc.vector.tensor_tensor(
                    out=outT_sb[:, h, :], in0=ot, in1=rb, op=Mult
                )

            # ============= phase 3: to_out partial =========================
            # fo shares the st_psum slots (same shape) so Wo accumulation can
            # begin as soon as the last exp frees an ST slot. Heads 1..7 are
            # accumulated in the per-oc loop; head 0 (whose outT is ready
            # first) finishes each oc one iteration later.
            with tc.tile_pool(name="fout", bufs=3) as fout_pool:
                def finish_oc(oc, fo):
                    for half in range(2):
                        nc.tensor.matmul(
                            fo[:, half * 512 : (half + 1) * 512],
                            woT_sb[:, 0, oc * 128 : (oc + 1) * 128],
                            outT_sb[:, 0, half * 512 : (half + 1) * 512],
                            start=False,
                            stop=True,
                        )
                    fout = fout_pool.tile([128, N], BF16)
                    nc.scalar.activation(out=fout, in_=fo, func=Copy)
                    for sh in range(2):
                        eng = nc.sync if sh == 0 else nc.scalar
                        eng.dma_start(
                            out=outT_d[oc * 128 : (oc + 1) * 128,
                                       sh * 512 : (sh + 1) * 512],
                            in_=fout[:, sh * 512 : (sh + 1) * 512],
                        )

                pending_oc = None
                for oc in range(16):
                    fo = st_psum.tile([128, N], F32, tag="st")
                    for h in range(1, HPC):
                        for half in range(2):
                            nc.tensor.matmul(
                                fo[:, half * 512 : (half + 1) * 512],
                                woT_sb[:, h, oc * 128 : (oc + 1) * 128],
                                outT_sb[:, h, half * 512 : (half + 1) * 512],
                                start=(h == 1),
                                stop=False,
                            )
                    if pending_oc is not None:
                        finish_oc(*pending_oc)
                    pending_oc = (oc, fo)
                finish_oc(*pending_oc)

            for p in reversed(attn_pools):
                p.__exit__(None, None, None)


_CACHE = {}


def _get_module():
    if "nc" in _CACHE:
        return _CACHE["nc"]
    nc = bacc.Bacc("TRN2", target_bir_lowering=False, debug=False, num_devices=8)
    xTd = nc.dram_tensor("xT", (D, N), BF16, kind="ExternalInput").ap()
    wq_d = nc.dram_tensor("wqT", (C, C), BF16, kind="ExternalInput").ap()
    wk_d = nc.dram_tensor("wkT", (C, C), BF16, kind="ExternalInput").ap()
    wv_d = nc.dram_tensor("wvT", (C, C), BF16, kind="ExternalInput").ap()
    bq_d = nc.dram_tensor("bq", (C, 1), F32, kind="ExternalInput").ap()
    bk_d = nc.dram_tensor("bk", (C, 1), F32, kind="ExternalInput").ap()
    bv_d = nc.dram_tensor("bv", (C, 1), F32, kind="ExternalInput").ap()
    wtc_d = nc.dram_tensor("wtc", (C, 1), BF16, kind="ExternalInput").ap()
    wo_d = nc.dram_tensor("woT", (HPC * C, D), BF16, kind="ExternalInput").ap()
    outT_d = nc.dram_tensor("outT", (D, N), BF16, kind="ExternalOutput").ap()

    with tile.TileContext(nc) as tc:
        _body(tc, xTd, wq_d, wk_d, wv_d, bq_d, bk_d, bv_d, wtc_d, wo_d, outT_d)
    nc.compile()
    _CACHE["nc"] = nc
    return nc


def make_in_maps(x, Wq, bq, Wk, bk, Wv, bv, Wl, bl, Wo, bo):
    x = np.asarray(x, np.float32)
    Wq = np.asarray(Wq, np.float32)
    Wk = np.asarray(Wk, np.float32)
    Wv = np.asarray(Wv, np.float32)
    Wl = np.asarray(Wl, np.float32)
    Wo = np.asarray(Wo, np.float32)
    we = (Wl[0] @ Wq) / float(NCHUNK)  # (128,) logits weight per chunk
    bf = ml_dtypes.bfloat16
    common = {
        "wqT": np.ascontiguousarray(Wq.T).astype(bf),
        "wkT": np.ascontiguousarray(Wk.T).astype(bf),
        "wvT": np.ascontiguousarray(Wv.T).astype(bf),
        "bq": np.asarray(bq, np.float32).reshape(C, 1),
        "bk": np.asarray(bk, np.float32).reshape(C, 1),
        "bv": np.asarray(bv, np.float32).reshape(C, 1),
        "wtc": we.reshape(C, 1).astype(bf),
    }
    woT = np.ascontiguousarray(Wo.T)  # (d, o)
    woT_half = [
        woT[0:1024, :].astype(bf),
        woT[1024:2048, :].astype(bf),
    ]
    in_maps = []
    xT_whole = [np.ascontiguousarray(x[b].T).astype(bf) for b in range(B)]
    for core in range(8):
        b, g = divmod(core, 2)
        xbT = xT_whole[b]
        xcore = xbT if g == 0 else np.ascontiguousarray(
            np.concatenate([xbT[1024:], xbT[:1024]], axis=0)
        )
        in_maps.append({"xT": xcore, "woT": woT_half[g], **common})
    return in_maps


def run_spmd(in_maps, trace=False, **kw):
    nc = _get_module()
    return bass_utils.run_bass_kernel_spmd(
        nc, in_maps, core_ids=list(range(8)), trace=trace, **kw
    )


def gather(results, bo):
    bo = np.asarray(bo, np.float32)
    out = np.empty((B, N, D), np.float32)
    for b in range(B):
        p0 = results[2 * b]["outT"].astype(np.float32).T
        p1 = results[2 * b + 1]["outT"].astype(np.float32).T
        out[b] = p0 + p1 + bo
    return out


def kernel(x, Wq, bq, Wk, bk, Wv, bv, Wl, bl, Wo, bo, stage=None, **_unused):
    in_maps = make_in_maps(x, Wq, bq, Wk, bk, Wv, bv, Wl, bl, Wo, bo)
    try:
        res = run_spmd(in_maps)
    except Exception:
        # transient device/runtime hiccup: retry once after a short pause
        import time as _time

        _time.sleep(2.0)
        res = run_spmd(in_maps)
    return gather(res.results, bo)
